# revision 41
# baseline (speedup 1.0000x reference)
"""Multi-head attention (B=8, S=1500, E=1024, H=16, D=64) on 8 trn2 NeuronCores.

Sharding: pure data-parallel over batch — core b computes batch element b
end-to-end (no collectives). Host pre-transposes x and the weights so every
device-side matmul has its contraction dim on the SBUF partition axis, and
folds the 1/sqrt(D) scale into Wq/bq and the V-bias into the output bias
(bo_eff = bo + Wo @ bv), so the device kernel never touches bv.

Device pipeline per core (all f32 storage, matmuls run as float32r):
  QT = (Wq/8)^T-proj of x^T   [1024, 1500]  (f-on-partition; bias bq/8 per-partition)
  KT = Wk^T-proj              [1024, 1500]
  V_aug = x @ Wv^T with a ones-column appended per head  [1500, 16*65]
  per (i-chunk, head): scoresT[j, i] via matmul(lhsT=KT_h, rhs=QT_h);
    exp on ACT (no max-subtraction: |scores| <~ 30, safely inside f32);
    causal masking structurally (affine_select zero-fill on diagonal blocks)
    or via an additive mask tensor (general path);
  out^T + rowsums in ONE matmul: lhsT=[V_h | 1] (65 cols), rhs=attnT;
  normalize: recip of the sums row, rank-1 matmul broadcast across 64
    partitions, multiply on eviction -> AO^T;
  yT = Wo^T-proj of AO^T + bo_eff  -> DRAM [1024, 1500], host transposes back.
"""

import sys
import numpy as np
import ml_dtypes

for _p in ("/opt/trn_rl_repo",):
    if _p not in sys.path:
        sys.path.append(_p)

import concourse.bass as bass
import concourse.mybir as mybir
import concourse.tile as tile
from concourse import bacc
from concourse.bass_utils import run_bass_kernel_spmd

F32 = mybir.dt.float32

B, S, E, H, D = 8, 1500, 1024, 16, 64
P = 128
NEG = -1e9


def _chunks(total, step):
    return [(c0, min(step, total - c0)) for c0 in range(0, total, step)]


def _wslices(dram_ap, col0, cols):
    """[E, E] weight -> [P, E//P, cols] AP for a column slice (k on partition)."""
    return dram_ap.rearrange("(kt p) f -> p kt f", p=P)[:, :, col0:col0 + cols]


def build(causal: bool, mm_dt=mybir.dt.bfloat16):
    KT_N = E // P            # k-tiles over the embedding dim
    FT_N = E // P            # f-tiles
    R_CH = _chunks(S, 512)   # i/r chunks
    JB = _chunks(S, P)       # j blocks
    H_LOC = E // D
    nc = bacc.Bacc("TRN2", target_bir_lowering=False, debug=False, num_devices=8)
    MD = mm_dt  # dtype for every matmul operand chain
    NRM = mybir.dt.float32r if MD == mybir.dt.bfloat16 else MD

    xT = nc.dram_tensor("xT", [E, S], MD, kind="ExternalInput")
    wqT = nc.dram_tensor("wqT", [E, E], MD, kind="ExternalInput")
    wkT = nc.dram_tensor("wkT", [E, E], MD, kind="ExternalInput")
    wvT = nc.dram_tensor("wvT", [E, E], MD, kind="ExternalInput")
    woT = nc.dram_tensor("woT", [E, E], MD, kind="ExternalInput")
    bq = nc.dram_tensor("bq", [E], F32, kind="ExternalInput")
    bo = nc.dram_tensor("bo", [E], F32, kind="ExternalInput")
    maskT = None
    if not causal:
        maskT = nc.dram_tensor("maskT", [S, S], F32, kind="ExternalInput")
    yT = nc.dram_tensor("yT", [E, S], F32, kind="ExternalOutput")

    def mm(ap):
        return ap

    nc._allow_low_precision_reason = "low-precision matmul operand chain"
    with tile.TileContext(nc) as tc:
        with (
            tc.tile_pool(name="persist", bufs=1) as pers,
            tc.tile_pool(name="wqkp", bufs=2) as wqkp,
            tc.tile_pool(name="wvp", bufs=1) as wvp,
            tc.tile_pool(name="wop", bufs=1) as wop,
            tc.tile_pool(name="attn", bufs=3) as apool,
            tc.tile_pool(name="small", bufs=3) as spool,
            tc.tile_pool(name="evp", bufs=3) as evp,
            tc.tile_pool(name="psP", bufs=1, space="PSUM") as psP,
            tc.tile_pool(name="psS", bufs=1, space="PSUM") as psS,
            tc.tile_pool(name="psO", bufs=1, space="PSUM") as psO,
            tc.tile_pool(name="psY", bufs=1, space="PSUM") as psY,
        ):
            ones64 = pers.tile([1, D], NRM, name="ones64")
            nc.vector.memset(ones64[:].bitcast(F32), 1.0)
            # K=33 selection weight: row 0 -> psum partitions 0-63, row 32 ->
            # 64-127 (fp32r matmuls require dst partition base 0; engine
            # accesses must start on 32-aligned partitions, hence the gap)
            selP = pers.tile([33, P], NRM, name="selP")
            nc.vector.memset(selP[:].bitcast(F32), 0.0)
            nc.vector.memset(selP[0:1, 0:D].bitcast(F32), 1.0)
            nc.vector.memset(selP[32:33, D:P].bitcast(F32), 1.0)
            ssumP_bufs = [pers.tile([33, 512], NRM, name=f"ssump{i}")
                          for i in range(3)]
            for _t in ssumP_bufs:
                nc.vector.memset(_t[:].bitcast(F32), 0.0)
            norm_seq = [0]
            bq_sb = pers.tile([P, FT_N], F32, name="bq_sb")
            nc.sync.dma_start(out=bq_sb[:], in_=bq.ap().rearrange("(t p) -> p t", p=P))
            bo_sb = pers.tile([P, FT_N], F32, name="bo_sb")
            nc.sync.dma_start(out=bo_sb[:], in_=bo.ap().rearrange("(t p) -> p t", p=P))

            # upper-triangular (incl diag) 0/1 mask for diagonal attn blocks
            tri32 = pers.tile([P, P], F32, name="tri32")
            nc.gpsimd.memset(tri32[:], 1.0)
            nc.gpsimd.affine_select(
                out=tri32[:], in_=tri32[:],
                pattern=[[1, P]], compare_op=mybir.AluOpType.is_ge,
                fill=0.0, base=0, channel_multiplier=-1,
            )
            tri = pers.tile([P, P], MD, name="tri")
            nc.vector.tensor_copy(out=tri[:], in_=tri32[:])

            XT = [pers.tile([P, S], MD, name=f"xt{kt}") for kt in range(KT_N)]
            QT = [pers.tile([P, S], MD, name=f"qt{ft}") for ft in range(FT_N)]
            KTs = [pers.tile([P, S], MD, name=f"kt{ft}") for ft in range(FT_N)]
            VA = [pers.tile([P, H_LOC * (D + 1)], MD, name=f"va{rt}")
                  for rt in range(len(JB))]
            AOT = [pers.tile([P, S], MD, name=f"aot{ft}") for ft in range(FT_N)]

            # ---- V projection (natural layout, ones column appended) ----
            hpc = 512 // D  # heads per 512-wide f chunk
            fchunks = _chunks(E, 512)
            wv_tiles = [wvp.tile([P, KT_N, 512], MD, name=f"wv{fc}", tag=f"wv{fc}")
                        for fc in range(len(fchunks))]
            # interleave wv-slice and xT-tile loads so the first V matmuls
            # can issue as soon as (wv[:,0,:], xT[0]) land
            for kt in range(KT_N):
                for fc, (f0, fw) in enumerate(fchunks):
                    nc.sync.dma_start(
                        out=wv_tiles[fc][:, kt, :fw],
                        in_=_wslices(wvT.ap(), f0, fw)[:, kt, :])
                nc.sync.dma_start(out=XT[kt][:],
                                  in_=xT[kt * P:(kt + 1) * P, :])

            def emit_v(rts):
                for rt in rts:
                    if rt >= len(JB):
                        continue
                    r0, rsz = JB[rt]
                    for fc, (f0, fw) in enumerate(fchunks):
                        wt = wv_tiles[fc]
                        ps = psP.tile([P, 512], F32, name="pv", tag="pp", bufs=2)
                        for kt in range(KT_N):
                            nc.tensor.matmul(
                                ps[:rsz, :fw],
                                mm(XT[kt][:, r0:r0 + rsz]),
                                mm(wt[:, kt, :fw]),
                                start=(kt == 0), stop=(kt == KT_N - 1),
                            )
                        dst = VA[rt][:].rearrange("p (h c) -> p h c", c=D + 1)
                        nc.vector.tensor_copy(
                            out=dst[:rsz, fc * hpc:fc * hpc + fw // D, 0:D],
                            in_=ps[:rsz, :fw].rearrange("p (h d) -> p h d", d=D),
                        )
                    va3 = VA[rt][:].rearrange("p (h c) -> p h c", c=D + 1)
                    if MD == mybir.dt.float32r:
                        nc.gpsimd.memset(va3[:rsz, :, D:D + 1].bitcast(F32), 1.0)
                    else:
                        nc.gpsimd.memset(va3[:rsz, :, D:D + 1], 1.0)

            def proj_qk_gen(ft):
                for which, wdram, dst in (("q", wqT, QT), ("k", wkT, KTs)):
                    wt = wqkp.tile([P, KT_N, P], MD, name="wqk", tag="wqk")
                    nc.sync.dma_start(out=wt[:], in_=_wslices(wdram.ap(), ft * P, P))
                    for rc, (c0, cw) in enumerate(R_CH):
                        ps = psP.tile([P, 512], F32, name="pp", tag="pp", bufs=2)
                        for kt in range(KT_N):
                            nc.tensor.matmul(
                                ps[:, :cw],
                                mm(wt[:, kt, :]),
                                mm(XT[kt][:, c0:c0 + cw]),
                                start=(kt == 0), stop=(kt == KT_N - 1),
                            )
                        if which == "q":
                            nc.vector.tensor_scalar(
                                out=dst[ft][:, c0:c0 + cw], in0=ps[:, :cw],
                                scalar1=bq_sb[:, ft:ft + 1], scalar2=None,
                                op0=mybir.AluOpType.add,
                            )
                        else:
                            nc.vector.tensor_copy(
                                out=dst[ft][:, c0:c0 + cw], in_=ps[:, :cw])
                        yield

            def proj_qk(ft):
                for _ in proj_qk_gen(ft):
                    pass

            def attn_ft(ic, ft, mtiles, filler=None, chunk=None):
                c0, cw = chunk if chunk is not None else R_CH[ic]
                nblk = (min(len(JB), (c0 + cw + P - 1) // P)
                        if causal else len(JB))
                pso = [psO.tile([D + 1, 512], F32, name=f"po{half}",
                                tag="po", bufs=2)
                       for half in range(2)]
                # diagonal-containing blocks first so the chunk-end attnV
                # gates on a short (non-masked) exp chain
                if causal:
                    cut = max(0, nblk - (cw + P - 1) // P)
                    order = list(range(cut, nblk)) + list(range(cut))
                else:
                    order = list(range(nblk))
                for n_i, jb in enumerate(order):
                    j0, jsz = JB[jb]
                    vo = max(0, j0 - c0) if causal else 0
                    # both halves' scores land in one 2-bank psum pair so a
                    # single ACTIVATE exps them together (halves ACT op count)
                    psp = psS.tile([P, 2, 512], F32, name="psp",
                                   tag="ps", bufs=2)
                    for half in range(2):
                        d0 = D * half
                        nc.tensor.matmul(
                            psp[:jsz, half, vo:cw],
                            mm(KTs[ft][d0:d0 + D, j0:j0 + jsz]),
                            mm(QT[ft][d0:d0 + D, c0 + vo:c0 + cw]),
                            start=True, stop=True,
                            tile_position=(d0, 0),
                        )
                    if not causal:
                        for half in range(2):
                            nc.vector.tensor_tensor(
                                out=psp[:jsz, half, :cw],
                                in0=psp[:jsz, half, :cw],
                                in1=mtiles[jb][:jsz, :cw],
                                op=mybir.AluOpType.add,
                            )
                    atp = apool.tile([P, 2, 512], MD, name="atp")
                    nc.scalar.activation(
                        out=atp[:jsz, :, vo:cw], in_=psp[:jsz, :, vo:cw],
                        func=mybir.ActivationFunctionType.Exp,
                    )
                    if causal and j0 >= c0:
                        # zero attn where j > i on the diagonal square
                        for half in range(2):
                            nc.vector.tensor_tensor(
                                out=atp[:jsz, half, vo:vo + jsz],
                                in0=atp[:jsz, half, vo:vo + jsz],
                                in1=tri[:jsz, :jsz],
                                op=mybir.AluOpType.mult,
                            )
                    va3 = VA[jb][:].rearrange("p (h c) -> p h c", c=D + 1)
                    for half in range(2):
                        nc.tensor.matmul(
                            pso[half][:, vo:cw],
                            mm(va3[:jsz, 2 * ft + half, :]),
                            mm(atp[:jsz, half, vo:cw]),
                            start=(n_i == 0), stop=(n_i == nblk - 1),
                        )
                    if filler is not None and n_i % 4 == 3:
                        filler()
                # both halves' sums land in rows 0/32 of a pre-zeroed [33, .]
                # tile; one K=33 matmul broadcasts them to psum partitions
                # 0-63 / 64-127, one reciprocal serves both normalize mults
                ssumP = ssumP_bufs[norm_seq[0] % len(ssumP_bufs)]
                norm_seq[0] += 1
                for half in range(2):
                    r0 = 32 * half
                    nc.vector.tensor_copy(
                        out=ssumP[r0:r0 + 1, :cw],
                        in_=pso[half][D:D + 1, :cw])
                psb = psP.tile([P, 512], F32, name="psb", tag="pp", bufs=2)
                nc.tensor.matmul(
                    psb[:, :cw], mm(selP[:, :]), mm(ssumP[:, :cw]),
                    start=True, stop=True,
                )
                rb = spool.tile([P, 512], F32, name="rb")
                nc.vector.reciprocal_approx_fast(
                    out=rb[:, :cw], in_=psb[:, :cw])
                for half in range(2):
                    d0 = D * half
                    nc.vector.tensor_tensor(
                        out=AOT[ft][d0:d0 + D, c0:c0 + cw],
                        in0=pso[half][0:D, :cw], in1=rb[d0:d0 + D, :cw],
                        op=mybir.AluOpType.mult,
                    )

            def emit_yt(ot, rc, wo_t, dmaq=None, chunk=None):
                c0, cw = chunk if chunk is not None else R_CH[rc]
                psy = psP.tile([P, 512], F32, name="py", tag="pp", bufs=2)
                for ft in range(FT_N):
                    nc.tensor.matmul(
                        psy[:, :cw],
                        mm(wo_t[:, ft, :]),
                        mm(AOT[ft][:, c0:c0 + cw]),
                        start=(ft == 0), stop=(ft == FT_N - 1),
                    )
                yt = evp.tile([P, 512], F32, name="yt", tag="yt")
                nc.vector.tensor_scalar(
                    out=yt[:, :cw], in0=psy[:, :cw],
                    scalar1=bo_sb[:, ot:ot + 1], scalar2=None,
                    op0=mybir.AluOpType.add,
                )
                (dmaq or nc.sync).dma_start(
                    out=yT[ot * P:(ot + 1) * P, c0:c0 + cw], in_=yt[:, :cw])

            if causal:
                wo_tiles = []
                for ot in range(FT_N):
                    wt = wop.tile([P, KT_N, P], MD, name=f"wo{ot}",
                                  tag=f"wo{ot}")
                    nc.sync.dma_start(out=wt[:],
                                      in_=_wslices(woT.ap(), ot * P, P))
                    wo_tiles.append(wt)
                nb0 = min(len(JB), (R_CH[0][0] + R_CH[0][1] + P - 1) // P)
                emit_v(range(nb0))
                proj_qk(0)
                nbp = nb0
                for ft in range(FT_N):
                    gen = proj_qk_gen(ft + 1) if ft + 1 < FT_N else None

                    def pump():
                        if gen is not None:
                            next(gen, None)

                    for ic in range(len(R_CH)):
                        attn_ft(ic, ft, None, filler=pump)
                        if ft == 0 and ic + 1 < len(R_CH):
                            c0n, cwn = R_CH[ic + 1]
                            nbn = min(len(JB), (c0n + cwn + P - 1) // P)
                            emit_v(range(nbp, nbn))
                            nbp = nbn
                        if ft == FT_N - 1:
                            # last ft has no next-ft projection filler: use the
                            # now-ready yT chunk as PE filler instead
                            for ot in range(FT_N):
                                emit_yt(ot, ic, wo_tiles[ot])
                    if gen is not None:
                        for _ in gen:
                            pass
            else:
                emit_v(range(len(JB)))
                for ft in range(FT_N):
                    proj_qk(ft)
                with tc.tile_pool(name="maskp", bufs=1) as mpool:
                    for ic, (c0, cw) in enumerate(R_CH):
                        mtiles = []
                        for jb, (j0, jsz) in enumerate(JB):
                            mt = mpool.tile([P, 512], F32, name=f"m{jb}")
                            nc.sync.dma_start(
                                out=mt[:jsz, :cw],
                                in_=maskT[j0:j0 + jsz, c0:c0 + cw])
                            mtiles.append(mt)
                        for ft in range(FT_N):
                            attn_ft(ic, ft, mtiles)
                for ot in range(FT_N):
                    wt = wop.tile([P, KT_N, P], MD, name=f"wo{ot}", tag="wo",
                                  bufs=2)
                    nc.sync.dma_start(out=wt[:], in_=_wslices(woT.ap(), ot * P, P))
                    for rc in range(len(R_CH)):
                        emit_yt(ot, rc, wt)

    nc.compile()
    return nc


_CACHE: dict = {}


def _get_nc(causal: bool):
    if causal not in _CACHE:
        _CACHE[causal] = build(causal)
    return _CACHE[causal]


def _is_causal(mask: np.ndarray) -> bool:
    if mask.shape != (S, S):
        return False
    expect = np.where(np.tril(np.ones((S, S), dtype=bool)), np.float32(0.0),
                      np.float32(NEG))
    return bool(np.array_equal(mask, expect))


MM_NP = ml_dtypes.bfloat16  # numpy dtype matching build()'s default mm_dt


def unpack_y(yT_dev):
    """Device yT [E, S] -> [S, E] float32."""
    return np.ascontiguousarray(
        np.asarray(yT_dev).reshape(E, S).T.astype(np.float32))


def prep_inputs(x, mask, Wq, bq, Wk, Wv, bv, Wo, bo):
    """Host-side preprocessing shared by kernel() and the bench harness."""
    scale = np.float32(1.0 / np.sqrt(D))
    xT = np.ascontiguousarray(np.transpose(x, (0, 2, 1)).astype(np.float32)).astype(MM_NP)
    common = {
        "wqT": np.ascontiguousarray((Wq.astype(np.float32) * scale).T).astype(MM_NP),
        "wkT": np.ascontiguousarray(Wk.astype(np.float32).T).astype(MM_NP),
        "wvT": np.ascontiguousarray(Wv.astype(np.float32).T).astype(MM_NP),
        "woT": np.ascontiguousarray(Wo.astype(np.float32).T).astype(MM_NP),
        "bq": (bq.astype(np.float32) * scale),
        "bo": (bo.astype(np.float32) + Wo.astype(np.float32) @ bv.astype(np.float32)),
    }
    causal = _is_causal(np.asarray(mask))
    if not causal:
        common["maskT"] = np.ascontiguousarray(np.asarray(mask, np.float32).T)
    in_maps = [dict(common, xT=xT[b]) for b in range(B)]
    return causal, in_maps


_RUNNER: dict = {}


def _get_runner(causal: bool):
    """Compile once per mask-variant; cache the jitted SPMD executable."""
    if causal in _RUNNER:
        return _RUNNER[causal]
    import jax
    from jax.sharding import Mesh, PartitionSpec, NamedSharding
    import warnings
    with warnings.catch_warnings():
        warnings.simplefilter("ignore")
        from jax.experimental.shard_map import shard_map
    from concourse import bass2jax
    from concourse.bass2jax import _bass_exec_p, install_neuronx_cc_hook

    nc = _get_nc(causal)
    install_neuronx_cc_hook()
    partition_name = (nc.partition_id_tensor.name
                      if nc.partition_id_tensor else None)
    in_names, out_names, out_avals = [], [], []
    for alloc in nc.m.functions[0].allocations:
        if not isinstance(alloc, mybir.MemoryLocationSet):
            continue
        name = alloc.memorylocations[0].name
        if alloc.kind == "ExternalInput":
            if name != partition_name:
                in_names.append(name)
        elif alloc.kind == "ExternalOutput":
            out_names.append(name)
            out_avals.append(jax.core.ShapedArray(
                tuple(alloc.tensor_shape), mybir.dt.np(alloc.dtype)))
    n_params = len(in_names)
    n_outs = len(out_names)

    def _body(*args):
        operands = list(args)
        names = list(in_names) + list(out_names)
        if partition_name is not None:
            operands.append(bass2jax.partition_id_tensor())
            names.append(partition_name)
        outs = _bass_exec_p.bind(
            *operands,
            out_avals=tuple(out_avals),
            in_names=tuple(names),
            out_names=tuple(out_names),
            lowering_input_output_aliases=(),
            sim_require_finite=True,
            sim_require_nnan=True,
            nc=nc,
        )
        return tuple(outs)

    devices = jax.devices()[:B]
    mesh = Mesh(np.asarray(devices), ("core",))
    in_specs = (PartitionSpec("core"),) * (n_params + n_outs)
    out_specs = (PartitionSpec("core"),) * n_outs
    fn = jax.jit(
        shard_map(_body, mesh=mesh, in_specs=in_specs, out_specs=out_specs,
                  check_rep=False),
        donate_argnums=tuple(range(n_params, n_params + n_outs)),
        keep_unused=True,
    )
    runner = (fn, in_names, out_names, out_avals)
    _RUNNER[causal] = runner
    return runner


def kernel(x, mask, Wq, bq, Wk, Wv, bv, Wo, bo):
    causal, in_maps = prep_inputs(x, mask, Wq, bq, Wk, Wv, bv, Wo, bo)
    fn, in_names, out_names, out_avals = _get_runner(causal)
    cat = [np.concatenate([np.asarray(m[n]) for m in in_maps], axis=0)
           for n in in_names]
    zs = [np.zeros((B * a.shape[0], *a.shape[1:]), a.dtype) for a in out_avals]
    outs = fn(*cat, *zs)
    yT = np.asarray(outs[out_names.index("yT")]).reshape(B, E, S)
    out = np.ascontiguousarray(yT.transpose(0, 2, 1).astype(np.float32))
    return out



# revision 43
# speedup vs baseline: 1.0004x; 1.0004x over previous
"""Multi-head attention (B=8, S=1500, E=1024, H=16, D=64) on 8 trn2 NeuronCores.

Sharding: pure data-parallel over batch — core b computes batch element b
end-to-end (no collectives). Host pre-transposes x and the weights so every
device-side matmul has its contraction dim on the SBUF partition axis, and
folds the 1/sqrt(D) scale into Wq/bq and the V-bias into the output bias
(bo_eff = bo + Wo @ bv), so the device kernel never touches bv.

Device pipeline per core (all f32 storage, matmuls run as float32r):
  QT = (Wq/8)^T-proj of x^T   [1024, 1500]  (f-on-partition; bias bq/8 per-partition)
  KT = Wk^T-proj              [1024, 1500]
  V_aug = x @ Wv^T with a ones-column appended per head  [1500, 16*65]
  per (i-chunk, head): scoresT[j, i] via matmul(lhsT=KT_h, rhs=QT_h);
    exp on ACT (no max-subtraction: |scores| <~ 30, safely inside f32);
    causal masking structurally (affine_select zero-fill on diagonal blocks)
    or via an additive mask tensor (general path);
  out^T + rowsums in ONE matmul: lhsT=[V_h | 1] (65 cols), rhs=attnT;
  normalize: recip of the sums row, rank-1 matmul broadcast across 64
    partitions, multiply on eviction -> AO^T;
  yT = Wo^T-proj of AO^T + bo_eff  -> DRAM [1024, 1500], host transposes back.
"""

import sys
import numpy as np
import ml_dtypes

for _p in ("/opt/trn_rl_repo",):
    if _p not in sys.path:
        sys.path.append(_p)

import concourse.bass as bass
import concourse.mybir as mybir
import concourse.tile as tile
from concourse import bacc
from concourse.bass_utils import run_bass_kernel_spmd

F32 = mybir.dt.float32

B, S, E, H, D = 8, 1500, 1024, 16, 64
P = 128
NEG = -1e9


def _chunks(total, step):
    return [(c0, min(step, total - c0)) for c0 in range(0, total, step)]


def _wslices(dram_ap, col0, cols):
    """[E, E] weight -> [P, E//P, cols] AP for a column slice (k on partition)."""
    return dram_ap.rearrange("(kt p) f -> p kt f", p=P)[:, :, col0:col0 + cols]


def build(causal: bool, mm_dt=mybir.dt.bfloat16):
    KT_N = E // P            # k-tiles over the embedding dim
    FT_N = E // P            # f-tiles
    R_CH = _chunks(S, 512)   # i/r chunks
    JB = _chunks(S, P)       # j blocks
    H_LOC = E // D
    nc = bacc.Bacc("TRN2", target_bir_lowering=False, debug=False, num_devices=8)
    MD = mm_dt  # dtype for every matmul operand chain
    NRM = mybir.dt.float32r if MD == mybir.dt.bfloat16 else MD

    xT = nc.dram_tensor("xT", [E, S], MD, kind="ExternalInput")
    wqT = nc.dram_tensor("wqT", [E, E], MD, kind="ExternalInput")
    wkT = nc.dram_tensor("wkT", [E, E], MD, kind="ExternalInput")
    wvT = nc.dram_tensor("wvT", [E, E], MD, kind="ExternalInput")
    woT = nc.dram_tensor("woT", [E, E], MD, kind="ExternalInput")
    bq = nc.dram_tensor("bq", [E], F32, kind="ExternalInput")
    bo = nc.dram_tensor("bo", [E], F32, kind="ExternalInput")
    maskT = None
    if not causal:
        maskT = nc.dram_tensor("maskT", [S, S], F32, kind="ExternalInput")
    # output in bf16: halves the 6 MB store traffic (the tail's critical
    # path); bf16 rounding adds <=0.2% to a 0.32% error vs a 2% gate
    yT = nc.dram_tensor("yT", [E, S], MD, kind="ExternalOutput")

    def mm(ap):
        return ap

    nc._allow_low_precision_reason = "low-precision matmul operand chain"
    with tile.TileContext(nc) as tc:
        with (
            tc.tile_pool(name="persist", bufs=1) as pers,
            tc.tile_pool(name="wqkp", bufs=2) as wqkp,
            tc.tile_pool(name="wvp", bufs=1) as wvp,
            tc.tile_pool(name="wop", bufs=1) as wop,
            tc.tile_pool(name="attn", bufs=3) as apool,
            tc.tile_pool(name="small", bufs=3) as spool,
            tc.tile_pool(name="evp", bufs=3) as evp,
            tc.tile_pool(name="psP", bufs=1, space="PSUM") as psP,
            tc.tile_pool(name="psS", bufs=1, space="PSUM") as psS,
            tc.tile_pool(name="psO", bufs=1, space="PSUM") as psO,
            tc.tile_pool(name="psY", bufs=1, space="PSUM") as psY,
        ):
            ones64 = pers.tile([1, D], NRM, name="ones64")
            nc.vector.memset(ones64[:].bitcast(F32), 1.0)
            # K=33 selection weight: row 0 -> psum partitions 0-63, row 32 ->
            # 64-127 (fp32r matmuls require dst partition base 0; engine
            # accesses must start on 32-aligned partitions, hence the gap)
            selP = pers.tile([33, P], NRM, name="selP")
            nc.vector.memset(selP[:].bitcast(F32), 0.0)
            nc.vector.memset(selP[0:1, 0:D].bitcast(F32), 1.0)
            nc.vector.memset(selP[32:33, D:P].bitcast(F32), 1.0)
            ssumP_bufs = [pers.tile([33, 512], NRM, name=f"ssump{i}")
                          for i in range(3)]
            for _t in ssumP_bufs:
                nc.vector.memset(_t[:].bitcast(F32), 0.0)
            norm_seq = [0]
            bq_sb = pers.tile([P, FT_N], F32, name="bq_sb")
            nc.sync.dma_start(out=bq_sb[:], in_=bq.ap().rearrange("(t p) -> p t", p=P))
            bo_sb = pers.tile([P, FT_N], F32, name="bo_sb")
            nc.sync.dma_start(out=bo_sb[:], in_=bo.ap().rearrange("(t p) -> p t", p=P))

            # upper-triangular (incl diag) 0/1 mask for diagonal attn blocks
            tri32 = pers.tile([P, P], F32, name="tri32")
            nc.gpsimd.memset(tri32[:], 1.0)
            nc.gpsimd.affine_select(
                out=tri32[:], in_=tri32[:],
                pattern=[[1, P]], compare_op=mybir.AluOpType.is_ge,
                fill=0.0, base=0, channel_multiplier=-1,
            )
            tri = pers.tile([P, P], MD, name="tri")
            nc.vector.tensor_copy(out=tri[:], in_=tri32[:])

            XT = [pers.tile([P, S], MD, name=f"xt{kt}") for kt in range(KT_N)]
            QT = [pers.tile([P, S], MD, name=f"qt{ft}") for ft in range(FT_N)]
            KTs = [pers.tile([P, S], MD, name=f"kt{ft}") for ft in range(FT_N)]
            VA = [pers.tile([P, H_LOC * (D + 1)], MD, name=f"va{rt}")
                  for rt in range(len(JB))]
            AOT = [pers.tile([P, S], MD, name=f"aot{ft}") for ft in range(FT_N)]

            # ---- V projection (natural layout, ones column appended) ----
            hpc = 512 // D  # heads per 512-wide f chunk
            fchunks = _chunks(E, 512)
            wv_tiles = [wvp.tile([P, KT_N, 512], MD, name=f"wv{fc}", tag=f"wv{fc}")
                        for fc in range(len(fchunks))]
            # interleave wv-slice and xT-tile loads so the first V matmuls
            # can issue as soon as (wv[:,0,:], xT[0]) land
            for kt in range(KT_N):
                for fc, (f0, fw) in enumerate(fchunks):
                    nc.sync.dma_start(
                        out=wv_tiles[fc][:, kt, :fw],
                        in_=_wslices(wvT.ap(), f0, fw)[:, kt, :])
                nc.sync.dma_start(out=XT[kt][:],
                                  in_=xT[kt * P:(kt + 1) * P, :])

            def emit_v(rts):
                for rt in rts:
                    if rt >= len(JB):
                        continue
                    r0, rsz = JB[rt]
                    for fc, (f0, fw) in enumerate(fchunks):
                        wt = wv_tiles[fc]
                        ps = psP.tile([P, 512], F32, name="pv", tag="pp", bufs=2)
                        for kt in range(KT_N):
                            nc.tensor.matmul(
                                ps[:rsz, :fw],
                                mm(XT[kt][:, r0:r0 + rsz]),
                                mm(wt[:, kt, :fw]),
                                start=(kt == 0), stop=(kt == KT_N - 1),
                            )
                        dst = VA[rt][:].rearrange("p (h c) -> p h c", c=D + 1)
                        nc.vector.tensor_copy(
                            out=dst[:rsz, fc * hpc:fc * hpc + fw // D, 0:D],
                            in_=ps[:rsz, :fw].rearrange("p (h d) -> p h d", d=D),
                        )
                    va3 = VA[rt][:].rearrange("p (h c) -> p h c", c=D + 1)
                    if MD == mybir.dt.float32r:
                        nc.gpsimd.memset(va3[:rsz, :, D:D + 1].bitcast(F32), 1.0)
                    else:
                        nc.gpsimd.memset(va3[:rsz, :, D:D + 1], 1.0)

            def proj_qk_gen(ft):
                for which, wdram, dst in (("q", wqT, QT), ("k", wkT, KTs)):
                    wt = wqkp.tile([P, KT_N, P], MD, name="wqk", tag="wqk")
                    nc.sync.dma_start(out=wt[:], in_=_wslices(wdram.ap(), ft * P, P))
                    for rc, (c0, cw) in enumerate(R_CH):
                        ps = psP.tile([P, 512], F32, name="pp", tag="pp", bufs=2)
                        for kt in range(KT_N):
                            nc.tensor.matmul(
                                ps[:, :cw],
                                mm(wt[:, kt, :]),
                                mm(XT[kt][:, c0:c0 + cw]),
                                start=(kt == 0), stop=(kt == KT_N - 1),
                            )
                        if which == "q":
                            nc.vector.tensor_scalar(
                                out=dst[ft][:, c0:c0 + cw], in0=ps[:, :cw],
                                scalar1=bq_sb[:, ft:ft + 1], scalar2=None,
                                op0=mybir.AluOpType.add,
                            )
                        else:
                            nc.vector.tensor_copy(
                                out=dst[ft][:, c0:c0 + cw], in_=ps[:, :cw])
                        yield

            def proj_qk(ft):
                for _ in proj_qk_gen(ft):
                    pass

            def attn_ft(ic, ft, mtiles, filler=None, chunk=None):
                c0, cw = chunk if chunk is not None else R_CH[ic]
                nblk = (min(len(JB), (c0 + cw + P - 1) // P)
                        if causal else len(JB))
                pso = [psO.tile([D + 1, 512], F32, name=f"po{half}",
                                tag="po", bufs=2)
                       for half in range(2)]
                # diagonal-containing blocks first so the chunk-end attnV
                # gates on a short (non-masked) exp chain
                if causal:
                    cut = max(0, nblk - (cw + P - 1) // P)
                    order = list(range(cut, nblk)) + list(range(cut))
                else:
                    order = list(range(nblk))
                for n_i, jb in enumerate(order):
                    j0, jsz = JB[jb]
                    vo = max(0, j0 - c0) if causal else 0
                    # both halves' scores land in one 2-bank psum pair so a
                    # single ACTIVATE exps them together (halves ACT op count)
                    psp = psS.tile([P, 2, 512], F32, name="psp",
                                   tag="ps", bufs=2)
                    for half in range(2):
                        d0 = D * half
                        nc.tensor.matmul(
                            psp[:jsz, half, vo:cw],
                            mm(KTs[ft][d0:d0 + D, j0:j0 + jsz]),
                            mm(QT[ft][d0:d0 + D, c0 + vo:c0 + cw]),
                            start=True, stop=True,
                            tile_position=(d0, 0),
                        )
                    if not causal:
                        for half in range(2):
                            nc.vector.tensor_tensor(
                                out=psp[:jsz, half, :cw],
                                in0=psp[:jsz, half, :cw],
                                in1=mtiles[jb][:jsz, :cw],
                                op=mybir.AluOpType.add,
                            )
                    atp = apool.tile([P, 2, 512], MD, name="atp")
                    nc.scalar.activation(
                        out=atp[:jsz, :, vo:cw], in_=psp[:jsz, :, vo:cw],
                        func=mybir.ActivationFunctionType.Exp,
                    )
                    if causal and j0 >= c0:
                        # zero attn where j > i on the diagonal square
                        for half in range(2):
                            nc.vector.tensor_tensor(
                                out=atp[:jsz, half, vo:vo + jsz],
                                in0=atp[:jsz, half, vo:vo + jsz],
                                in1=tri[:jsz, :jsz],
                                op=mybir.AluOpType.mult,
                            )
                    va3 = VA[jb][:].rearrange("p (h c) -> p h c", c=D + 1)
                    for half in range(2):
                        nc.tensor.matmul(
                            pso[half][:, vo:cw],
                            mm(va3[:jsz, 2 * ft + half, :]),
                            mm(atp[:jsz, half, vo:cw]),
                            start=(n_i == 0), stop=(n_i == nblk - 1),
                        )
                    if filler is not None and n_i % 4 == 3:
                        filler()
                # both halves' sums land in rows 0/32 of a pre-zeroed [33, .]
                # tile; one K=33 matmul broadcasts them to psum partitions
                # 0-63 / 64-127, one reciprocal serves both normalize mults
                ssumP = ssumP_bufs[norm_seq[0] % len(ssumP_bufs)]
                norm_seq[0] += 1
                for half in range(2):
                    r0 = 32 * half
                    nc.vector.tensor_copy(
                        out=ssumP[r0:r0 + 1, :cw],
                        in_=pso[half][D:D + 1, :cw])
                psb = psP.tile([P, 512], F32, name="psb", tag="pp", bufs=2)
                nc.tensor.matmul(
                    psb[:, :cw], mm(selP[:, :]), mm(ssumP[:, :cw]),
                    start=True, stop=True,
                )
                rb = spool.tile([P, 512], F32, name="rb")
                nc.vector.reciprocal_approx_fast(
                    out=rb[:, :cw], in_=psb[:, :cw])
                for half in range(2):
                    d0 = D * half
                    nc.vector.tensor_tensor(
                        out=AOT[ft][d0:d0 + D, c0:c0 + cw],
                        in0=pso[half][0:D, :cw], in1=rb[d0:d0 + D, :cw],
                        op=mybir.AluOpType.mult,
                    )

            def emit_yt(ot, rc, wo_t, dmaq=None, chunk=None):
                c0, cw = chunk if chunk is not None else R_CH[rc]
                psy = psP.tile([P, 512], F32, name="py", tag="pp", bufs=2)
                for ft in range(FT_N):
                    nc.tensor.matmul(
                        psy[:, :cw],
                        mm(wo_t[:, ft, :]),
                        mm(AOT[ft][:, c0:c0 + cw]),
                        start=(ft == 0), stop=(ft == FT_N - 1),
                    )
                yt = evp.tile([P, 512], MD, name="yt", tag="yt")
                nc.vector.tensor_scalar(
                    out=yt[:, :cw], in0=psy[:, :cw],
                    scalar1=bo_sb[:, ot:ot + 1], scalar2=None,
                    op0=mybir.AluOpType.add,
                )
                (dmaq or nc.sync).dma_start(
                    out=yT[ot * P:(ot + 1) * P, c0:c0 + cw], in_=yt[:, :cw])

            if causal:
                wo_tiles = []
                for ot in range(FT_N):
                    wt = wop.tile([P, KT_N, P], MD, name=f"wo{ot}",
                                  tag=f"wo{ot}")
                    nc.sync.dma_start(out=wt[:],
                                      in_=_wslices(woT.ap(), ot * P, P))
                    wo_tiles.append(wt)
                nb0 = min(len(JB), (R_CH[0][0] + R_CH[0][1] + P - 1) // P)
                emit_v(range(nb0))
                proj_qk(0)
                nbp = nb0
                for ft in range(FT_N):
                    gen = proj_qk_gen(ft + 1) if ft + 1 < FT_N else None

                    def pump():
                        if gen is not None:
                            next(gen, None)

                    for ic in range(len(R_CH)):
                        attn_ft(ic, ft, None, filler=pump)
                        if ft == 0 and ic + 1 < len(R_CH):
                            c0n, cwn = R_CH[ic + 1]
                            nbn = min(len(JB), (c0n + cwn + P - 1) // P)
                            emit_v(range(nbp, nbn))
                            nbp = nbn
                        if ft == FT_N - 1:
                            # last ft has no next-ft projection filler: use the
                            # now-ready yT chunk as PE filler instead
                            for ot in range(FT_N):
                                emit_yt(ot, ic, wo_tiles[ot])
                    if gen is not None:
                        for _ in gen:
                            pass
            else:
                emit_v(range(len(JB)))
                for ft in range(FT_N):
                    proj_qk(ft)
                with tc.tile_pool(name="maskp", bufs=1) as mpool:
                    for ic, (c0, cw) in enumerate(R_CH):
                        mtiles = []
                        for jb, (j0, jsz) in enumerate(JB):
                            mt = mpool.tile([P, 512], F32, name=f"m{jb}")
                            nc.sync.dma_start(
                                out=mt[:jsz, :cw],
                                in_=maskT[j0:j0 + jsz, c0:c0 + cw])
                            mtiles.append(mt)
                        for ft in range(FT_N):
                            attn_ft(ic, ft, mtiles)
                for ot in range(FT_N):
                    wt = wop.tile([P, KT_N, P], MD, name=f"wo{ot}", tag="wo",
                                  bufs=2)
                    nc.sync.dma_start(out=wt[:], in_=_wslices(woT.ap(), ot * P, P))
                    for rc in range(len(R_CH)):
                        emit_yt(ot, rc, wt)

    nc.compile()
    return nc


_CACHE: dict = {}


def _get_nc(causal: bool):
    if causal not in _CACHE:
        _CACHE[causal] = build(causal)
    return _CACHE[causal]


def _is_causal(mask: np.ndarray) -> bool:
    if mask.shape != (S, S):
        return False
    expect = np.where(np.tril(np.ones((S, S), dtype=bool)), np.float32(0.0),
                      np.float32(NEG))
    return bool(np.array_equal(mask, expect))


MM_NP = ml_dtypes.bfloat16  # numpy dtype matching build()'s default mm_dt


def unpack_y(yT_dev):
    """Device yT [E, S] -> [S, E] float32."""
    return np.ascontiguousarray(
        np.asarray(yT_dev).reshape(E, S).T.astype(np.float32))


def prep_inputs(x, mask, Wq, bq, Wk, Wv, bv, Wo, bo):
    """Host-side preprocessing shared by kernel() and the bench harness."""
    scale = np.float32(1.0 / np.sqrt(D))
    xT = np.ascontiguousarray(np.transpose(x, (0, 2, 1)).astype(np.float32)).astype(MM_NP)
    common = {
        "wqT": np.ascontiguousarray((Wq.astype(np.float32) * scale).T).astype(MM_NP),
        "wkT": np.ascontiguousarray(Wk.astype(np.float32).T).astype(MM_NP),
        "wvT": np.ascontiguousarray(Wv.astype(np.float32).T).astype(MM_NP),
        "woT": np.ascontiguousarray(Wo.astype(np.float32).T).astype(MM_NP),
        "bq": (bq.astype(np.float32) * scale),
        "bo": (bo.astype(np.float32) + Wo.astype(np.float32) @ bv.astype(np.float32)),
    }
    causal = _is_causal(np.asarray(mask))
    if not causal:
        common["maskT"] = np.ascontiguousarray(np.asarray(mask, np.float32).T)
    in_maps = [dict(common, xT=xT[b]) for b in range(B)]
    return causal, in_maps


_RUNNER: dict = {}


def _get_runner(causal: bool):
    """Compile once per mask-variant; cache the jitted SPMD executable."""
    if causal in _RUNNER:
        return _RUNNER[causal]
    import jax
    from jax.sharding import Mesh, PartitionSpec, NamedSharding
    import warnings
    with warnings.catch_warnings():
        warnings.simplefilter("ignore")
        from jax.experimental.shard_map import shard_map
    from concourse import bass2jax
    from concourse.bass2jax import _bass_exec_p, install_neuronx_cc_hook

    nc = _get_nc(causal)
    install_neuronx_cc_hook()
    partition_name = (nc.partition_id_tensor.name
                      if nc.partition_id_tensor else None)
    in_names, out_names, out_avals = [], [], []
    for alloc in nc.m.functions[0].allocations:
        if not isinstance(alloc, mybir.MemoryLocationSet):
            continue
        name = alloc.memorylocations[0].name
        if alloc.kind == "ExternalInput":
            if name != partition_name:
                in_names.append(name)
        elif alloc.kind == "ExternalOutput":
            out_names.append(name)
            out_avals.append(jax.core.ShapedArray(
                tuple(alloc.tensor_shape), mybir.dt.np(alloc.dtype)))
    n_params = len(in_names)
    n_outs = len(out_names)

    def _body(*args):
        operands = list(args)
        names = list(in_names) + list(out_names)
        if partition_name is not None:
            operands.append(bass2jax.partition_id_tensor())
            names.append(partition_name)
        outs = _bass_exec_p.bind(
            *operands,
            out_avals=tuple(out_avals),
            in_names=tuple(names),
            out_names=tuple(out_names),
            lowering_input_output_aliases=(),
            sim_require_finite=True,
            sim_require_nnan=True,
            nc=nc,
        )
        return tuple(outs)

    devices = jax.devices()[:B]
    mesh = Mesh(np.asarray(devices), ("core",))
    in_specs = (PartitionSpec("core"),) * (n_params + n_outs)
    out_specs = (PartitionSpec("core"),) * n_outs
    fn = jax.jit(
        shard_map(_body, mesh=mesh, in_specs=in_specs, out_specs=out_specs,
                  check_rep=False),
        donate_argnums=tuple(range(n_params, n_params + n_outs)),
        keep_unused=True,
    )
    runner = (fn, in_names, out_names, out_avals)
    _RUNNER[causal] = runner
    return runner


def kernel(x, mask, Wq, bq, Wk, Wv, bv, Wo, bo):
    causal, in_maps = prep_inputs(x, mask, Wq, bq, Wk, Wv, bv, Wo, bo)
    fn, in_names, out_names, out_avals = _get_runner(causal)
    cat = [np.concatenate([np.asarray(m[n]) for m in in_maps], axis=0)
           for n in in_names]
    zs = [np.zeros((B * a.shape[0], *a.shape[1:]), a.dtype) for a in out_avals]
    outs = fn(*cat, *zs)
    yT = np.asarray(outs[out_names.index("yT")]).reshape(B, E, S)
    out = np.ascontiguousarray(yT.transpose(0, 2, 1).astype(np.float32))
    return out

